# revision 2
# baseline (speedup 1.0000x reference)
"""Trainium2 Bass kernel for GCN+RNN (nn_GCNN_RNN_32461362823865).

Strategy:
  - Host: build dense normalized adjacency A^T (fp16, 3072-padded) from the
    edge list (exact reference remap semantics), fold W2 = W @ W_ih.T and
    c0 = b @ W_ih.T + b_ih + b_hh, pre-transpose/cast x_in per core.
  - Device phase 1 (batch-sharded, 16 samples/core):
      z = x @ W2                  (128->50, PE, fp16)
      U^T = (z^T stationary) x A^T streamed  -> (s*50+j, dst_node) f32 PSUM
      cast fp16, write to AllToAll input laid out dest-core-major.
  - AllToAll: reshard batch-sharded U to node-sharded (384 nodes/core).
  - Device phase 3 (node-sharded RNN): h^T (50, 384) fp16; per step one
    matmul with stacked stationary [W_hh^T; I] (K=100) accumulating
    W_hh-recurrence + U[b], then ScalarE tanh with per-partition bias c0.
  - Host: concat per-core (B, 50, 384) outputs, transpose, crop, upcast.
"""
import numpy as np

import concourse.bacc as bacc
import concourse.mybir as mybir
from concourse import tile
from concourse.bass_utils import run_bass_kernel_spmd

# ---- problem constants (hardcoded per contract) ----
N = 3070          # nodes
NP = 3072         # padded nodes (24 * 128, 8 * 384)
F = 128           # input features
J = 50            # folded feature dim (= RNN hidden)
B = 128           # batch (RNN sequence length)
NCORES = 8
S = B // NCORES   # samples per core = 16
NPC = NP // NCORES  # nodes per core = 384
SJ = S * J        # 800 rows of U^T per core
KB = NP // 128    # 24 contraction blocks
MB = (SJ + 127) // 128  # 7 M-blocks (last has 32 rows)

F16 = mybir.dt.float16
F32 = mybir.dt.float32

_PROGRAM_CACHE = {}


def _build_program():
    if "nc" in _PROGRAM_CACHE:
        return _PROGRAM_CACHE["nc"]
    nc = bacc.Bacc("TRN2", target_bir_lowering=False, debug=False,
                   num_devices=NCORES)

    xT = nc.dram_tensor("xT", [S, F, N], F16, kind="ExternalInput")
    at = nc.dram_tensor("at", [NP, NP], F16, kind="ExternalInput")
    w2 = nc.dram_tensor("w2", [F, J], F16, kind="ExternalInput")
    ws = nc.dram_tensor("ws", [2 * J, J], F16, kind="ExternalInput")
    c0 = nc.dram_tensor("c0", [J, 1], F32, kind="ExternalInput")
    h0T = nc.dram_tensor("h0T", [J, NPC], F16, kind="ExternalInput")
    out = nc.dram_tensor("out", [B, J, NPC], F16, kind="ExternalOutput")

    with tile.TileContext(nc) as tc:
        with (
            tc.tile_pool(name="persist", bufs=1) as persist,
            tc.tile_pool(name="consts", bufs=1) as consts,
            tc.tile_pool(name="dram", bufs=1, space="DRAM") as dram,
        ):
            # persistent SBUF tensors
            at_sb = persist.tile([128, KB * NP], F16, tag="at_sb")
            z_sb = persist.tile([128, KB * SJ], F16, tag="z_sb")
            w2_sb = consts.tile([F, J], F16, tag="w2_sb")
            ws_sb = consts.tile([2 * J, J], F16, tag="ws_sb")
            c0_sb = consts.tile([J, 1], F32, tag="c0_sb")

            nc.sync.dma_start(w2_sb[:], w2[:])
            nc.sync.dma_start(ws_sb[:], ws[:])
            nc.sync.dma_start(c0_sb[:], c0[:])
            for kb in range(KB):
                nc.sync.dma_start(at_sb[:, kb * NP:(kb + 1) * NP],
                                  at[kb * 128:(kb + 1) * 128, :])

            a2a_in = dram.tile([NCORES * SJ, NPC], F16)
            a2a_out = dram.tile([NCORES * SJ, NPC], F16)

            # ---- phase 1a: z[src, (s,j)] = x @ W2, per 128-node block ----
            with nc.named_scope("zphase"):
                with (
                    tc.tile_pool(name="xin", bufs=4) as xin,
                    tc.tile_pool(name="zpsum", bufs=4, space="PSUM") as zpsum,
                ):
                    for kb in range(KB):
                        ncols = min(128, N - kb * 128)  # 126 for last block
                        for s in range(S):
                            xt = xin.tile([F, 128], F16, tag="xt")
                            if ncols < 128:
                                nc.vector.memset(xt[:], 0.0)
                            nc.sync.dma_start(
                                xt[:, 0:ncols],
                                xT[s, :, kb * 128:kb * 128 + ncols])
                            zp = zpsum.tile([128, J], F32, tag="zp")
                            nc.tensor.matmul(zp[:], xt[:], w2_sb[:],
                                             start=True, stop=True)
                            nc.scalar.activation(
                                z_sb[:, kb * SJ + s * J: kb * SJ + (s + 1) * J],
                                zp[:], mybir.ActivationFunctionType.Copy)

            # ---- phase 1b: U^T = z^T x A^T  (M=sj rows, N=dst nodes) ----
            with nc.named_scope("ummphase"):
                with (
                    tc.tile_pool(name="upsum", bufs=8, space="PSUM") as upsum,
                    tc.tile_pool(name="stg", bufs=4) as stg,
                ):
                    for mb in range(MB):
                        mrows = min(128, SJ - mb * 128)
                        psums = []
                        for dc in range(NCORES):
                            psums.append(upsum.tile([mrows, NPC], F32, tag="up", name=f"up_{mb}_{dc}"))
                        for kb in range(KB):
                            lhsT = z_sb[:, kb * SJ + mb * 128:
                                        kb * SJ + mb * 128 + mrows]
                            for dc in range(NCORES):
                                nc.tensor.matmul(
                                    psums[dc][:], lhsT,
                                    at_sb[:, kb * NP + dc * NPC:
                                          kb * NP + (dc + 1) * NPC],
                                    start=(kb == 0), stop=(kb == KB - 1))
                        for dc in range(NCORES):
                            st = stg.tile([mrows, NPC], F16, tag="st")
                            nc.scalar.activation(
                                st[:], psums[dc][:],
                                mybir.ActivationFunctionType.Copy)
                            nc.sync.dma_start(
                                a2a_in[dc * SJ + mb * 128:
                                       dc * SJ + mb * 128 + mrows, :],
                                st[:])

            # ---- phase 2: AllToAll reshard (batch-shard -> node-shard) ----
            with nc.named_scope("a2a"):
                nc.gpsimd.collective_compute(
                    "AllToAll", mybir.AluOpType.bypass,
                    replica_groups=[list(range(NCORES))],
                    ins=[a2a_in.opt()], outs=[a2a_out.opt()])

            # ---- phase 3: RNN over 128 steps, node-sharded ----
            with nc.named_scope("rnn"):
                with (
                    tc.tile_pool(name="rhs", bufs=1) as rhspool,
                    tc.tile_pool(name="p3psum", bufs=4, space="PSUM") as p3psum,
                ):
                    RING = 4
                    rhs = [rhspool.tile([2 * J, NPC], F16, tag=f"rhs{r}", name=f"rhs_{r}")
                           for r in range(RING)]
                    nc.sync.dma_start(rhs[0][0:J, :], h0T[:])
                    for b in range(B):
                        r = b % RING
                        nc.sync.dma_start(
                            rhs[r][J:2 * J, :],
                            a2a_out[b * J:(b + 1) * J, :])
                        pp = p3psum.tile([J, NPC], F32, tag="pp")
                        nc.tensor.matmul(pp[:], ws_sb[:], rhs[r][0:2 * J, :],
                                         start=True, stop=True)
                        nxt = rhs[(b + 1) % RING]
                        nc.scalar.activation(
                            nxt[0:J, :], pp[:],
                            mybir.ActivationFunctionType.Tanh,
                            bias=c0_sb[:, 0:1])
                        nc.sync.dma_start(out[b], nxt[0:J, :])

    nc.compile()
    _PROGRAM_CACHE["nc"] = nc
    return nc


def _host_prep(x_in, edge_index, edge_weight, W, b, W_ih, W_hh, b_ih, b_hh, h0):
    """Build per-core input maps (all numpy, no device work)."""
    edge_index = np.asarray(edge_index).astype(np.int64)
    # exact reference remap: rank among unique ids (size=N, fill=2**30)
    uniq = np.unique(edge_index)
    if uniq.size < N:
        uniq = np.concatenate([uniq, np.full(N - uniq.size, 2 ** 30, np.int64)])
    else:
        uniq = uniq[:N]
    ei = np.searchsorted(uniq, edge_index)
    src, dst = ei[0], ei[1]

    ew = np.asarray(edge_weight, np.float64)
    deg = np.zeros(N, np.float64)
    np.add.at(deg, dst, ew)
    deg += 1.0  # self loops (weight 1)
    dinv = np.where(deg > 0, 1.0 / np.sqrt(deg), 0.0)

    AT = np.zeros((NP, NP), np.float32)
    np.add.at(AT, (src, dst), (dinv[src] * ew * dinv[dst]).astype(np.float32))
    idx = np.arange(N)
    AT[idx, idx] += (dinv * dinv).astype(np.float32)
    AT16 = AT.astype(np.float16)

    W = np.asarray(W, np.float32)
    W_ih = np.asarray(W_ih, np.float32)
    W2 = (W.astype(np.float64) @ W_ih.T.astype(np.float64)).astype(np.float16)
    c0 = (np.asarray(b, np.float32) @ W_ih.T + np.asarray(b_ih, np.float32)
          + np.asarray(b_hh, np.float32)).astype(np.float32).reshape(J, 1)
    ws = np.concatenate([np.asarray(W_hh, np.float32).T, np.eye(J, dtype=np.float32)],
                        axis=0).astype(np.float16)

    x_in = np.asarray(x_in, np.float32)
    h0 = np.asarray(h0, np.float32)
    h0p = np.zeros((NP, J), np.float16)
    h0p[:N] = h0.astype(np.float16)

    in_maps = []
    for c in range(NCORES):
        xc = x_in[c * S:(c + 1) * S]                      # (S, N, F)
        xTc = np.ascontiguousarray(
            xc.transpose(0, 2, 1)).astype(np.float16)     # (S, F, N)
        h0Tc = np.ascontiguousarray(
            h0p[c * NPC:(c + 1) * NPC].T)                 # (J, NPC)
        in_maps.append({"xT": xTc, "at": AT16, "w2": W2, "ws": ws,
                        "c0": c0, "h0T": h0Tc})
    return in_maps


def _assemble(results):
    parts = []
    for c in range(NCORES):
        o = results[c]["out"]                 # (B, J, NPC) fp16
        parts.append(np.ascontiguousarray(o.transpose(0, 2, 1)))  # (B, NPC, J)
    full = np.concatenate(parts, axis=1)      # (B, NP, J)
    return full[:, :N, :].astype(np.float32)


def run_internal(inputs, trace=False, trace_cores=None):
    nc = _build_program()
    in_maps = _host_prep(**inputs)
    res = run_bass_kernel_spmd(nc, in_maps, list(range(NCORES)), trace=trace,
                               trace_cores=trace_cores)
    return _assemble(res.results), res


def kernel(**inputs) -> np.ndarray:
    out, _ = run_internal(inputs, trace=False)
    return out


# revision 6
# speedup vs baseline: 1.4818x; 1.4818x over previous
"""Trainium2 Bass kernel for GCN+RNN (nn_GCNN_RNN_32461362823865).

Strategy:
  - Host: build dense normalized adjacency A^T (fp16, 3072-padded) from the
    edge list (exact reference remap semantics), fold W2 = W @ W_ih.T and
    c0 = b @ W_ih.T + b_ih + b_hh, pre-transpose/cast x_in per core.
  - Device phase 1 (batch-sharded, 16 samples/core):
      z = x @ W2                  (128->50, PE, fp16)
      U^T = (z^T stationary) x A^T streamed  -> (s*50+j, dst_node) f32 PSUM
      cast fp16 (DVE), write to AllToAll input laid out dest-core-major.
  - AllToAll: reshard batch-sharded U to node-sharded (384 nodes/core).
  - Device phase 3 (node-sharded RNN): h^T (50, nodes) fp16; two node-chains
    (192 each) pipelined; per chain-step one matmul with stacked stationary
    [W_hh^T; I] (K=100) computing W_hh @ h + U[b], then ScalarE tanh with
    per-partition bias c0. U preloaded in 8 big DMAs; outputs staged in
    SBUF rings and written back 8 steps per DMA.
  - Host: concat per-core (B, 50, 384) outputs, transpose, crop, upcast.
"""
import numpy as np

import concourse.bacc as bacc
import concourse.mybir as mybir
from concourse import tile
from concourse.bass_utils import run_bass_kernel_spmd

# ---- problem constants (hardcoded per contract) ----
N = 3070          # nodes
NP = 3072         # padded nodes (24 * 128, 8 * 384)
F = 128           # input features
J = 50            # folded feature dim (= RNN hidden)
B = 128           # batch (RNN sequence length)
NCORES = 8
S = B // NCORES   # samples per core = 16
NPC = NP // NCORES  # nodes per core = 384
SJ = S * J        # 800 rows of U^T per core
KB = NP // 128    # 24 contraction blocks
MB = (SJ + 127) // 128  # 7 M-blocks (last has 32 rows)
HALF = NPC // 2   # 192 nodes per RNN chain

F16 = mybir.dt.float16
F32 = mybir.dt.float32
COPY = mybir.ActivationFunctionType.Copy
TANH = mybir.ActivationFunctionType.Tanh

_PROGRAM_CACHE = {}


def _build_program():
    if "nc" in _PROGRAM_CACHE:
        return _PROGRAM_CACHE["nc"]
    nc = bacc.Bacc("TRN2", target_bir_lowering=False, debug=False,
                   num_devices=NCORES)

    xT = nc.dram_tensor("xT", [S, F, N], F16, kind="ExternalInput")
    at = nc.dram_tensor("at", [NP, NP], F16, kind="ExternalInput")
    w2 = nc.dram_tensor("w2", [F, J], F16, kind="ExternalInput")
    ws = nc.dram_tensor("ws", [128, J], F16, kind="ExternalInput")
    c0 = nc.dram_tensor("c0", [J, 1], F32, kind="ExternalInput")
    h0T = nc.dram_tensor("h0T", [J, NPC], F16, kind="ExternalInput")
    out = nc.dram_tensor("out", [B, J, NPC], F16, kind="ExternalOutput")

    with tile.TileContext(nc) as tc:
        with (
            tc.tile_pool(name="consts", bufs=1) as consts,
            tc.tile_pool(name="dram", bufs=1, space="DRAM") as dram,
        ):
            w2_sb = consts.tile([F, J], F16, tag="w2_sb")
            ws_sb = consts.tile([128, J], F16, tag="ws_sb")
            c0_sb = consts.tile([J, 1], F32, tag="c0_sb")
            nc.sync.dma_start(w2_sb[:], w2[:])
            nc.sync.dma_start(ws_sb[:], ws[:])
            nc.sync.dma_start(c0_sb[:], c0[:])

            a2a_in = dram.tile([NCORES * SJ, NPC], F16)
            a2a_out = dram.tile([NCORES * SJ, NPC], F16)

            # ================= phase 1 (GCN as dense matmul) =============
            with tc.tile_pool(name="p12", bufs=1) as p12:
                at_sb = p12.tile([128, KB * NP], F16, tag="at_sb")
                z_sb = p12.tile([128, KB * SJ], F16, tag="z_sb")
                for kb in range(KB):
                    nc.sync.dma_start(at_sb[:, kb * NP:(kb + 1) * NP],
                                      at[kb * 128:(kb + 1) * 128, :])

                # ---- phase 1a: z[src,(s,j)] = x @ W2 ----
                with nc.named_scope("zphase"):
                    with (
                        tc.tile_pool(name="xin", bufs=2) as xin,
                        tc.tile_pool(name="zpsum", bufs=4, space="PSUM") as zpsum,
                    ):
                        for s in range(S):
                            xbig = xin.tile([F, NP], F16, tag="xbig")
                            nc.sync.dma_start(xbig[:, 0:N], xT[s])
                            nc.vector.memset(xbig[:, N:NP], 0.0)
                            for kb in range(KB):
                                zp = zpsum.tile([128, J], F32, tag="zp")
                                nc.tensor.matmul(
                                    zp[:], xbig[:, kb * 128:(kb + 1) * 128],
                                    w2_sb[:], start=True, stop=True)
                                nc.vector.tensor_copy(
                                    z_sb[:, kb * SJ + s * J:
                                         kb * SJ + (s + 1) * J], zp[:])

                # ---- phase 1b: U^T = z^T x A^T ----
                with nc.named_scope("ummphase"):
                    with (
                        tc.tile_pool(name="upsum", bufs=8, space="PSUM") as upsum,
                        tc.tile_pool(name="stg", bufs=4) as stg,
                    ):
                        for mb in range(MB):
                            mrows = min(128, SJ - mb * 128)
                            psums = []
                            for dc in range(NCORES):
                                psums.append(upsum.tile(
                                    [mrows, NPC], F32, tag="up",
                                    name=f"up_{mb}_{dc}"))
                            for kb in range(KB):
                                lhsT = z_sb[:, kb * SJ + mb * 128:
                                            kb * SJ + mb * 128 + mrows]
                                for dc in range(NCORES):
                                    nc.tensor.matmul(
                                        psums[dc][:], lhsT,
                                        at_sb[:, kb * NP + dc * NPC:
                                              kb * NP + (dc + 1) * NPC],
                                        start=(kb == 0), stop=(kb == KB - 1))
                            for dc in range(NCORES):
                                st = stg.tile([mrows, NPC], F16, tag="st")
                                nc.vector.tensor_copy(st[:], psums[dc][:])
                                nc.sync.dma_start(
                                    a2a_in[dc * SJ + mb * 128:
                                           dc * SJ + mb * 128 + mrows, :],
                                    st[:])

            # ============== phase 2: AllToAll reshard ====================
            with nc.named_scope("a2a"):
                nc.gpsimd.collective_compute(
                    "AllToAll", mybir.AluOpType.bypass,
                    replica_groups=[list(range(NCORES))],
                    ins=[a2a_in.opt()], outs=[a2a_out.opt()])

            # ============== phase 3: RNN (node-sharded) ==================
            with nc.named_scope("rnn"):
                with (
                    tc.tile_pool(name="p3", bufs=1) as p3,
                    tc.tile_pool(name="p3psum", bufs=4, space="PSUM") as p3psum,
                ):
                    # U preload: per source core c a (J, S*NPC) tile
                    u_big = []
                    for c in range(NCORES):
                        u = p3.tile([J, S * NPC], F16, tag=f"ubig{c}",
                                    name=f"ubig_{c}")
                        nc.sync.dma_start(
                            u[:].rearrange("j (s n) -> j s n", s=S),
                            a2a_out[c * SJ:(c + 1) * SJ, :].rearrange(
                                "(s j) n -> j s n", j=J))
                        u_big.append(u)

                    RING = 4
                    OUT_RING = 16
                    chains = []
                    for h in range(2):
                        stacks = [p3.tile([128, HALF], F16,
                                          tag=f"stk{h}{r}", name=f"stk_{h}_{r}")
                                  for r in range(RING)]
                        oring = p3.tile([J, OUT_RING * HALF], F16,
                                        tag=f"oring{h}", name=f"oring_{h}")
                        chains.append((stacks, oring))
                        nc.sync.dma_start(
                            stacks[0][0:J, :],
                            h0T[:, h * HALF:(h + 1) * HALF])

                    for b in range(B):
                        c, s = b // S, b % S
                        r = b % RING
                        slot = b % OUT_RING
                        for h, (stacks, oring) in enumerate(chains):
                            usrc = u_big[c][:, s * NPC + h * HALF:
                                            s * NPC + h * HALF + HALF]
                            nc.vector.tensor_copy(
                                stacks[r][64:64 + J, :], usrc)
                            pp = p3psum.tile([J, HALF], F32, tag=f"pp{h}",
                                             name=f"pp_{h}_{b}")
                            nc.tensor.matmul(pp[:], ws_sb[:],
                                             stacks[r][0:128, :],
                                             start=True, stop=True)
                            nxt = stacks[(b + 1) % RING]
                            nc.scalar.activation(nxt[0:J, :], pp[:], TANH,
                                                 bias=c0_sb[:, 0:1])
                            # stage h_{b+1} (= x_out[b]) for batched writeback
                            nc.vector.tensor_copy(
                                oring[:, slot * HALF:(slot + 1) * HALF],
                                nxt[0:J, :])
                        if slot % 8 == 7:
                            b0 = b - 7
                            for h, (stacks, oring) in enumerate(chains):
                                nc.sync.dma_start(
                                    out[b0:b0 + 8, :,
                                        h * HALF:(h + 1) * HALF].rearrange(
                                        "g j n -> j g n"),
                                    oring[:, (b0 % OUT_RING) * HALF:
                                          (b0 % OUT_RING + 8) * HALF].rearrange(
                                        "j (g n) -> j g n", g=8))

    nc.compile()
    _PROGRAM_CACHE["nc"] = nc
    return nc


def _host_prep(x_in, edge_index, edge_weight, W, b, W_ih, W_hh, b_ih, b_hh, h0):
    """Build per-core input maps (all numpy, no device work)."""
    edge_index = np.asarray(edge_index).astype(np.int64)
    # exact reference remap: rank among unique ids (size=N, fill=2**30)
    uniq = np.unique(edge_index)
    if uniq.size < N:
        uniq = np.concatenate([uniq, np.full(N - uniq.size, 2 ** 30, np.int64)])
    else:
        uniq = uniq[:N]
    ei = np.searchsorted(uniq, edge_index)
    src, dst = ei[0], ei[1]

    ew = np.asarray(edge_weight, np.float64)
    deg = np.zeros(N, np.float64)
    np.add.at(deg, dst, ew)
    deg += 1.0  # self loops (weight 1)
    dinv = np.where(deg > 0, 1.0 / np.sqrt(deg), 0.0)

    AT = np.zeros((NP, NP), np.float32)
    np.add.at(AT, (src, dst), (dinv[src] * ew * dinv[dst]).astype(np.float32))
    idx = np.arange(N)
    AT[idx, idx] += (dinv * dinv).astype(np.float32)
    AT16 = AT.astype(np.float16)

    W = np.asarray(W, np.float32)
    W_ih = np.asarray(W_ih, np.float32)
    W2 = (W.astype(np.float64) @ W_ih.T.astype(np.float64)).astype(np.float16)
    c0 = (np.asarray(b, np.float32) @ W_ih.T + np.asarray(b_ih, np.float32)
          + np.asarray(b_hh, np.float32)).astype(np.float32).reshape(J, 1)
    ws = np.zeros((128, J), np.float32)
    ws[0:J] = np.asarray(W_hh, np.float32).T
    ws[64:64 + J] = np.eye(J, dtype=np.float32)
    ws = ws.astype(np.float16)

    x_in = np.asarray(x_in, np.float32)
    h0 = np.asarray(h0, np.float32)
    h0p = np.zeros((NP, J), np.float16)
    h0p[:N] = h0.astype(np.float16)

    in_maps = []
    for c in range(NCORES):
        xc = x_in[c * S:(c + 1) * S]                      # (S, N, F)
        xTc = np.ascontiguousarray(
            xc.transpose(0, 2, 1)).astype(np.float16)     # (S, F, N)
        h0Tc = np.ascontiguousarray(
            h0p[c * NPC:(c + 1) * NPC].T)                 # (J, NPC)
        in_maps.append({"xT": xTc, "at": AT16, "w2": W2, "ws": ws,
                        "c0": c0, "h0T": h0Tc})
    return in_maps


def _assemble(results):
    parts = []
    for c in range(NCORES):
        o = results[c]["out"]                 # (B, J, NPC) fp16
        parts.append(np.ascontiguousarray(o.transpose(0, 2, 1)))  # (B, NPC, J)
    full = np.concatenate(parts, axis=1)      # (B, NP, J)
    return full[:, :N, :].astype(np.float32)


def run_internal(inputs, trace=False, trace_cores=None):
    nc = _build_program()
    in_maps = _host_prep(**inputs)
    res = run_bass_kernel_spmd(nc, in_maps, list(range(NCORES)), trace=trace,
                               trace_cores=trace_cores)
    return _assemble(res.results), res


def kernel(**inputs) -> np.ndarray:
    out, _ = run_internal(inputs, trace=False)
    return out


# revision 7
# speedup vs baseline: 1.5556x; 1.0498x over previous
"""Trainium2 Bass kernel for GCN+RNN (nn_GCNN_RNN_32461362823865).

Strategy:
  - Host: build dense normalized adjacency A^T (fp16, 3072-padded) from the
    edge list (exact reference remap semantics), fold W2 = W @ W_ih.T and
    c0 = b @ W_ih.T + b_ih + b_hh, pre-transpose/cast x_in per core.
  - Device phase 1 (batch-sharded, 16 samples/core):
      z = x @ W2                  (128->50, PE, fp16)
      U^T = (z^T stationary) x A^T streamed  -> (s*50+j, dst_node) f32 PSUM
      cast fp16 (DVE), write to AllToAll input laid out dest-core-major.
  - AllToAll: reshard batch-sharded U to node-sharded (384 nodes/core).
  - Device phase 3 (node-sharded RNN): h^T (50, nodes) fp16; two node-chains
    (192 each) pipelined; per chain-step one matmul with stacked stationary
    [W_hh^T; I] (K=100) computing W_hh @ h + U[b], then ScalarE tanh with
    per-partition bias c0. U preloaded in 8 big DMAs; outputs staged in
    SBUF rings and written back 8 steps per DMA.
  - Host: concat per-core (B, 50, 384) outputs, transpose, crop, upcast.
"""
import numpy as np

import concourse.bacc as bacc
import concourse.mybir as mybir
from concourse import tile
from concourse.bass_utils import run_bass_kernel_spmd

# ---- problem constants (hardcoded per contract) ----
N = 3070          # nodes
NP = 3072         # padded nodes (24 * 128, 8 * 384)
F = 128           # input features
J = 50            # folded feature dim (= RNN hidden)
B = 128           # batch (RNN sequence length)
NCORES = 8
S = B // NCORES   # samples per core = 16
NPC = NP // NCORES  # nodes per core = 384
SJ = S * J        # 800 rows of U^T per core
KB = NP // 128    # 24 contraction blocks
MB = (SJ + 127) // 128  # 7 M-blocks (last has 32 rows)
HALF = NPC // 2   # 192 nodes per RNN chain

F16 = mybir.dt.float16
F32 = mybir.dt.float32
COPY = mybir.ActivationFunctionType.Copy
TANH = mybir.ActivationFunctionType.Tanh

_PROGRAM_CACHE = {}


def _build_program():
    if "nc" in _PROGRAM_CACHE:
        return _PROGRAM_CACHE["nc"]
    nc = bacc.Bacc("TRN2", target_bir_lowering=False, debug=False,
                   num_devices=NCORES)

    xT = nc.dram_tensor("xT", [S, F, N], F16, kind="ExternalInput")
    at = nc.dram_tensor("at", [NP, NP], F16, kind="ExternalInput")
    w2 = nc.dram_tensor("w2", [F, J], F16, kind="ExternalInput")
    wh = nc.dram_tensor("wh", [J, J], F16, kind="ExternalInput")
    eye = nc.dram_tensor("eye", [J, J], F16, kind="ExternalInput")
    c0 = nc.dram_tensor("c0", [J, 1], F32, kind="ExternalInput")
    h0T = nc.dram_tensor("h0T", [J, NPC], F16, kind="ExternalInput")
    out = nc.dram_tensor("out", [B, J, NPC], F16, kind="ExternalOutput")

    with tile.TileContext(nc) as tc:
        with (
            tc.tile_pool(name="consts", bufs=1) as consts,
            tc.tile_pool(name="dram", bufs=1, space="DRAM") as dram,
        ):
            w2_sb = consts.tile([F, J], F16, tag="w2_sb")
            wh_sb = consts.tile([J, J], F16, tag="wh_sb")
            eye_sb = consts.tile([J, J], F16, tag="eye_sb")
            c0_sb = consts.tile([J, 1], F32, tag="c0_sb")
            nc.sync.dma_start(w2_sb[:], w2[:])
            nc.sync.dma_start(wh_sb[:], wh[:])
            nc.sync.dma_start(eye_sb[:], eye[:])
            nc.sync.dma_start(c0_sb[:], c0[:])

            a2a_in = dram.tile([NCORES * SJ, NPC], F16)
            a2a_out = dram.tile([NCORES * SJ, NPC], F16)

            # ================= phase 1 (GCN as dense matmul) =============
            with tc.tile_pool(name="p12", bufs=1) as p12:
                at_sb = p12.tile([128, KB * NP], F16, tag="at_sb")
                z_sb = p12.tile([128, KB * SJ], F16, tag="z_sb")

                # ---- phase 1a: z[src,(s,j)] = x @ W2 ----
                with nc.named_scope("zphase"):
                    with (
                        tc.tile_pool(name="xin", bufs=2) as xin,
                        tc.tile_pool(name="zpsum", bufs=4, space="PSUM") as zpsum,
                    ):
                        for s in range(S):
                            xbig = xin.tile([F, NP], F16, tag="xbig")
                            nc.sync.dma_start(xbig[:, 0:N], xT[s])
                            nc.vector.memset(xbig[:, N:NP], 0.0)
                            if s < 8:  # spread A^T loads behind the x loads
                                for kb in range(3 * s, 3 * s + 3):
                                    nc.sync.dma_start(
                                        at_sb[:, kb * NP:(kb + 1) * NP],
                                        at[kb * 128:(kb + 1) * 128, :])
                            for kb in range(KB):
                                zp = zpsum.tile([128, J], F32, tag="zp")
                                nc.tensor.matmul(
                                    zp[:], xbig[:, kb * 128:(kb + 1) * 128],
                                    w2_sb[:], start=True, stop=True)
                                nc.vector.tensor_copy(
                                    z_sb[:, kb * SJ + s * J:
                                         kb * SJ + (s + 1) * J], zp[:])

                # ---- phase 1b: U^T = z^T x A^T ----
                with nc.named_scope("ummphase"):
                    with (
                        tc.tile_pool(name="upsum", bufs=8, space="PSUM") as upsum,
                        tc.tile_pool(name="stg", bufs=4) as stg,
                    ):
                        for mb in range(MB):
                            mrows = min(128, SJ - mb * 128)
                            psums = []
                            for dc in range(NCORES):
                                psums.append(upsum.tile(
                                    [mrows, NPC], F32, tag="up",
                                    name=f"up_{mb}_{dc}"))
                            for kb in range(KB):
                                lhsT = z_sb[:, kb * SJ + mb * 128:
                                            kb * SJ + mb * 128 + mrows]
                                for dc in range(NCORES):
                                    nc.tensor.matmul(
                                        psums[dc][:], lhsT,
                                        at_sb[:, kb * NP + dc * NPC:
                                              kb * NP + (dc + 1) * NPC],
                                        start=(kb == 0), stop=(kb == KB - 1))
                            for dc in range(NCORES):
                                st = stg.tile([mrows, NPC], F16, tag="st")
                                nc.vector.tensor_copy(st[:], psums[dc][:])
                                nc.sync.dma_start(
                                    a2a_in[dc * SJ + mb * 128:
                                           dc * SJ + mb * 128 + mrows, :],
                                    st[:])

            # ============== phase 2: AllToAll reshard ====================
            with nc.named_scope("a2a"):
                nc.gpsimd.collective_compute(
                    "AllToAll", mybir.AluOpType.bypass,
                    replica_groups=[list(range(NCORES))],
                    ins=[a2a_in.opt()], outs=[a2a_out.opt()])

            # ============== phase 3: RNN (node-sharded) ==================
            with nc.named_scope("rnn"):
                with (
                    tc.tile_pool(name="p3", bufs=1) as p3,
                    tc.tile_pool(name="p3psum", bufs=4, space="PSUM") as p3psum,
                ):
                    # U tiles: per source core c a (J, S*NPC) tile,
                    # loads interleaved with the step loop for overlap
                    u_big = []
                    for c in range(NCORES):
                        u = p3.tile([J, S * NPC], F16, tag=f"ubig{c}",
                                    name=f"ubig_{c}")
                        u_big.append(u)

                    def load_ubig(c):
                        nc.sync.dma_start(
                            u_big[c][:].rearrange("j (s n) -> j s n", s=S),
                            a2a_out[c * SJ:(c + 1) * SJ, :].rearrange(
                                "(s j) n -> j s n", j=J))

                    load_ubig(0)
                    load_ubig(1)

                    RING = 4
                    OUT_RING = 16
                    chains = []
                    for h in range(2):
                        hts = [p3.tile([J, HALF], F16,
                                       tag=f"ht{h}{r}", name=f"ht_{h}_{r}")
                               for r in range(RING)]
                        oring = p3.tile([J, OUT_RING * HALF], F16,
                                        tag=f"oring{h}", name=f"oring_{h}")
                        chains.append((hts, oring))
                        nc.sync.dma_start(
                            hts[0][:], h0T[:, h * HALF:(h + 1) * HALF])

                    for b in range(B):
                        c, s = b // S, b % S
                        r = b % RING
                        slot = b % OUT_RING
                        if s == 0 and c + 2 < NCORES:
                            load_ubig(c + 2)
                        for h, (hts, oring) in enumerate(chains):
                            usrc = u_big[c][:, s * NPC + h * HALF:
                                            s * NPC + h * HALF + HALF]
                            pp = p3psum.tile([J, HALF], F32, tag=f"pp{h}",
                                             name=f"pp_{h}_{b}")
                            nc.tensor.matmul(pp[:], eye_sb[:], usrc,
                                             start=True, stop=False)
                            nc.tensor.matmul(pp[:], wh_sb[:], hts[r][:],
                                             start=False, stop=True)
                            nxt = hts[(b + 1) % RING]
                            nc.scalar.activation(nxt[:], pp[:], TANH,
                                                 bias=c0_sb[:, 0:1])
                            # stage h_{b+1} (= x_out[b]) for batched writeback
                            nc.vector.tensor_copy(
                                oring[:, slot * HALF:(slot + 1) * HALF],
                                nxt[:])
                        if slot % 8 == 7:
                            b0 = b - 7
                            for h, (hts, oring) in enumerate(chains):
                                nc.sync.dma_start(
                                    out[b0:b0 + 8, :,
                                        h * HALF:(h + 1) * HALF].rearrange(
                                        "g j n -> j g n"),
                                    oring[:, (b0 % OUT_RING) * HALF:
                                          (b0 % OUT_RING + 8) * HALF].rearrange(
                                        "j (g n) -> j g n", g=8))

    nc.compile()
    _PROGRAM_CACHE["nc"] = nc
    return nc


def _host_prep(x_in, edge_index, edge_weight, W, b, W_ih, W_hh, b_ih, b_hh, h0):
    """Build per-core input maps (all numpy, no device work)."""
    edge_index = np.asarray(edge_index).astype(np.int64)
    # exact reference remap: rank among unique ids (size=N, fill=2**30)
    uniq = np.unique(edge_index)
    if uniq.size < N:
        uniq = np.concatenate([uniq, np.full(N - uniq.size, 2 ** 30, np.int64)])
    else:
        uniq = uniq[:N]
    ei = np.searchsorted(uniq, edge_index)
    src, dst = ei[0], ei[1]

    ew = np.asarray(edge_weight, np.float64)
    deg = np.zeros(N, np.float64)
    np.add.at(deg, dst, ew)
    deg += 1.0  # self loops (weight 1)
    dinv = np.where(deg > 0, 1.0 / np.sqrt(deg), 0.0)

    AT = np.zeros((NP, NP), np.float32)
    np.add.at(AT, (src, dst), (dinv[src] * ew * dinv[dst]).astype(np.float32))
    idx = np.arange(N)
    AT[idx, idx] += (dinv * dinv).astype(np.float32)
    AT16 = AT.astype(np.float16)

    W = np.asarray(W, np.float32)
    W_ih = np.asarray(W_ih, np.float32)
    W2 = (W.astype(np.float64) @ W_ih.T.astype(np.float64)).astype(np.float16)
    c0 = (np.asarray(b, np.float32) @ W_ih.T + np.asarray(b_ih, np.float32)
          + np.asarray(b_hh, np.float32)).astype(np.float32).reshape(J, 1)
    wh = np.asarray(W_hh, np.float32).T.astype(np.float16)
    eye = np.eye(J, dtype=np.float16)

    x_in = np.asarray(x_in, np.float32)
    h0 = np.asarray(h0, np.float32)
    h0p = np.zeros((NP, J), np.float16)
    h0p[:N] = h0.astype(np.float16)

    in_maps = []
    for c in range(NCORES):
        xc = x_in[c * S:(c + 1) * S]                      # (S, N, F)
        xTc = np.ascontiguousarray(
            xc.transpose(0, 2, 1)).astype(np.float16)     # (S, F, N)
        h0Tc = np.ascontiguousarray(
            h0p[c * NPC:(c + 1) * NPC].T)                 # (J, NPC)
        in_maps.append({"xT": xTc, "at": AT16, "w2": W2, "wh": wh,
                        "eye": eye, "c0": c0, "h0T": h0Tc})
    return in_maps


def _assemble(results):
    parts = []
    for c in range(NCORES):
        o = results[c]["out"]                 # (B, J, NPC) fp16
        parts.append(np.ascontiguousarray(o.transpose(0, 2, 1)))  # (B, NPC, J)
    full = np.concatenate(parts, axis=1)      # (B, NP, J)
    return full[:, :N, :].astype(np.float32)


def run_internal(inputs, trace=False, trace_cores=None):
    nc = _build_program()
    in_maps = _host_prep(**inputs)
    res = run_bass_kernel_spmd(nc, in_maps, list(range(NCORES)), trace=trace,
                               trace_cores=trace_cores)
    return _assemble(res.results), res


def kernel(**inputs) -> np.ndarray:
    out, _ = run_internal(inputs, trace=False)
    return out


# revision 12
# speedup vs baseline: 1.6919x; 1.0876x over previous
"""Trainium2 Bass kernel for GCN+RNN (nn_GCNN_RNN_32461362823865).

Strategy:
  - Host: build dense normalized adjacency A^T (fp16, 3072-padded) from the
    edge list (exact reference remap semantics), fold W2 = W @ W_ih.T and
    c0 = b @ W_ih.T + b_ih + b_hh, pre-transpose/cast x_in per core.
  - Device phase 1 (batch-sharded, 16 samples/core in 4 rounds of 4):
      z = x @ W2                  (128->50, PE, fp16)
      U^T = (z^T stationary) x A^T streamed  -> (s*50+j, dst_node) f32 PSUM
      cast fp16 (DVE), write to per-round AllToAll input, dest-core-major.
  - 4 AllToAll rounds reshard U batch->node sharding (384 nodes/core);
    round r's collective overlaps round r+1's matmuls, and RNN steps for
    round r are woven into round r+2's matmul emission.
  - Phase 3 (node-sharded RNN): h^T (50, nodes) fp16; two node-chains
    (192 each) pipelined; per chain-step two matmuls (I x U accumulate
    W_hh^T x h) then ScalarE tanh with per-partition bias c0. Outputs
    staged in SBUF rings, written back 8 steps per DMA.
  - Host: concat per-core (B, 50, 384) outputs, transpose, crop, upcast.

  Sample->core map: core c, round r holds global samples 32*r + 4*c + [0..4).
"""
import numpy as np

import concourse.bacc as bacc
import concourse.mybir as mybir
from concourse import tile
from concourse.bass_utils import run_bass_kernel_spmd

# ---- problem constants (hardcoded per contract) ----
N = 3070          # nodes
NP = 3072         # padded nodes (24 * 128, 8 * 384)
F = 128           # input features
J = 50            # folded feature dim (= RNN hidden)
B = 128           # batch (RNN sequence length)
NCORES = 8
S = B // NCORES   # samples per core = 16
NPC = NP // NCORES  # nodes per core = 384
KB = NP // 128    # 24 contraction blocks
HALF = NPC // 2   # 192 nodes per RNN chain
R = 4             # a2a rounds
SR = S // R       # samples per round per core = 4
RJ = SR * J       # 200 U^T rows per round per core
MBS = [128, RJ - 128]   # M-block rows within a round
DCP = [list(range(6)), [6, 7]]  # dest-core passes (6+2 psum banks)

F16 = mybir.dt.float16
F32 = mybir.dt.float32
TANH = mybir.ActivationFunctionType.Tanh

_PROGRAM_CACHE = {}


def _build_program():
    if "nc" in _PROGRAM_CACHE:
        return _PROGRAM_CACHE["nc"]
    nc = bacc.Bacc("TRN2", target_bir_lowering=False, debug=False,
                   num_devices=NCORES)

    xT = nc.dram_tensor("xT", [S, F, N], F16, kind="ExternalInput")
    at = nc.dram_tensor("at", [NP, NP], F16, kind="ExternalInput")
    w2 = nc.dram_tensor("w2", [F, J], F16, kind="ExternalInput")
    wh = nc.dram_tensor("wh", [J, J], F16, kind="ExternalInput")
    eye = nc.dram_tensor("eye", [J, J], F16, kind="ExternalInput")
    c0 = nc.dram_tensor("c0", [J, 1], F32, kind="ExternalInput")
    h0T = nc.dram_tensor("h0T", [J, NPC], F16, kind="ExternalInput")
    out = nc.dram_tensor("out", [B, J, NPC], F16, kind="ExternalOutput")

    with tile.TileContext(nc) as tc:
        with (
            tc.tile_pool(name="consts", bufs=1) as consts,
            tc.tile_pool(name="persist", bufs=1) as persist,
            tc.tile_pool(name="dram", bufs=1, space="DRAM") as dram,
        ):
            w2_sb = consts.tile([F, J], F16, tag="w2_sb")
            wh_sb = consts.tile([J, J], F16, tag="wh_sb")
            eye_sb = consts.tile([J, J], F16, tag="eye_sb")
            c0_sb = consts.tile([J, 1], F32, tag="c0_sb")
            nc.sync.dma_start(w2_sb[:], w2[:])
            nc.sync.dma_start(wh_sb[:], wh[:])
            nc.sync.dma_start(eye_sb[:], eye[:])
            nc.sync.dma_start(c0_sb[:], c0[:])

            at_sb = persist.tile([128, KB * NP], F16, tag="at_sb")
            z_sb = persist.tile([128, KB * S * J], F16, tag="z_sb")
            SJ = S * J

            a2a_in = [dram.tile([NCORES * RJ, NPC], F16, name=f"a2ai_{r}")
                      for r in range(R)]
            a2a_out = [dram.tile([NCORES * RJ, NPC], F16, name=f"a2ao_{r}")
                       for r in range(R)]

            # ---- phase 1a: z[src,(s,j)] = x @ W2 (x loads first) ----
            with nc.named_scope("zphase"):
                with (
                    tc.tile_pool(name="xin", bufs=2) as xin,
                    tc.tile_pool(name="zpsum", bufs=2, space="PSUM") as zpsum,
                ):
                    for s in range(S):
                        xbig = xin.tile([F, NP], F16, tag="xbig")
                        nc.sync.dma_start(xbig[:, 0:N], xT[s])
                        nc.vector.memset(xbig[:, N:NP], 0.0)
                        for kb in range(KB):
                            zp = zpsum.tile([128, J], F32, tag="zp")
                            nc.tensor.matmul(
                                zp[:], xbig[:, kb * 128:(kb + 1) * 128],
                                w2_sb[:], start=True, stop=True)
                            nc.vector.tensor_copy(
                                z_sb[:, kb * SJ + s * J:
                                     kb * SJ + (s + 1) * J], zp[:])

            # A^T loads queue behind the x loads (needed from U-phase on)
            for kb in range(KB):
                nc.sync.dma_start(at_sb[:, kb * NP:(kb + 1) * NP],
                                  at[kb * 128:(kb + 1) * 128, :])

            # ---- RNN state (declared before weave) ----
            post0 = tc.tile_pool(name="upsum", bufs=6, space="PSUM")
            upsum = post0.__enter__()
            post0b = tc.tile_pool(name="p3psum", bufs=1, space="PSUM")
            p3psum = post0b.__enter__()
            post = tc.tile_pool(name="stg", bufs=2)
            stg = post.__enter__()
            post2 = tc.tile_pool(name="upool", bufs=2)
            upool = post2.__enter__()
            post3 = tc.tile_pool(name="p3", bufs=1)
            p3 = post3.__enter__()
            u_tiles = {}

            def load_ubig(r, c):
                u = upool.tile([J, SR * NPC], F16, tag="u",
                               name=f"ubig_{r}_{c}")
                u_tiles[(r, c)] = u
                nc.sync.dma_start(
                    u[:].rearrange("j (s n) -> j s n", s=SR),
                    a2a_out[r][c * RJ:(c + 1) * RJ, :].rearrange(
                        "(s j) n -> j s n", j=J))

            RING = 4
            OUT_RING = 12
            OGRP = 6
            chains = []
            for h in range(2):
                hts = [p3.tile([J, HALF], F16, name=f"ht_{h}_{r}")
                       for r in range(RING)]
                oring = p3.tile([J, OUT_RING * HALF], F16, name=f"oring_{h}")
                chains.append((hts, oring))
                nc.sync.dma_start(hts[0][:], h0T[:, h * HALF:(h + 1) * HALF])

            def rnn_step(b):
                r, c, s4 = b // 32, (b % 32) // 4, b % 4
                if s4 == 0 and c + 2 < NCORES:
                    load_ubig(r, c + 2)   # prefetch 2 source-cores ahead
                ring = b % RING
                slot = b % OUT_RING
                for h, (hts, oring) in enumerate(chains):
                    ut = u_tiles[(r, c)]
                    usrc = ut[:, s4 * NPC + h * HALF:
                              s4 * NPC + h * HALF + HALF]
                    pp = p3psum.tile([J, HALF], F32, tag=f"pp{h}",
                                     name=f"pp_{h}_{b}")
                    nc.tensor.matmul(pp[:], eye_sb[:], usrc,
                                     start=True, stop=False)
                    nc.tensor.matmul(pp[:], wh_sb[:], hts[ring][:],
                                     start=False, stop=True)
                    nxt = hts[(b + 1) % RING]
                    nc.scalar.activation(nxt[:], pp[:], TANH,
                                         bias=c0_sb[:, 0:1])
                    nc.vector.tensor_copy(
                        oring[:, slot * HALF:(slot + 1) * HALF], nxt[:])
                if slot % OGRP == OGRP - 1 and b >= OGRP - 1:
                    b0 = b - (OGRP - 1)
                    for h, (hts, oring) in enumerate(chains):
                        nc.sync.dma_start(
                            out[b0:b0 + OGRP, :,
                                h * HALF:(h + 1) * HALF].rearrange(
                                "g j n -> j g n"),
                            oring[:, (b0 % OUT_RING) * HALF:
                                  (b0 % OUT_RING + OGRP) * HALF].rearrange(
                                "j (g n) -> j g n", g=OGRP))

            # ---- phase 1b: U^T per round + collectives, weaving RNN ----
            with nc.named_scope("ummphase"):
                for r in range(R):
                    if r >= 2:  # u loads for round r-2 (a2a r-2 done by now)
                        load_ubig(r - 2, 0)
                        load_ubig(r - 2, 1)
                    weave = list(range(32 * (r - 2), 32 * (r - 2) + 32)) \
                        if r >= 2 else []
                    wi = 0
                    for mbi, mrows in enumerate(MBS):
                        row0 = r * RJ + mbi * 128  # start row in z cols
                        for pi, dcs in enumerate(DCP):
                            psums = {}
                            for dc in dcs:
                                psums[dc] = upsum.tile(
                                    [mrows, NPC], F32, tag="up",
                                    name=f"up_{r}_{mbi}_{dc}")
                            for kb in range(KB):
                                lhsT = z_sb[:, kb * SJ + row0:
                                            kb * SJ + row0 + mrows]
                                for dc in dcs:
                                    nc.tensor.matmul(
                                        psums[dc][:], lhsT,
                                        at_sb[:, kb * NP + dc * NPC:
                                              kb * NP + (dc + 1) * NPC],
                                        start=(kb == 0), stop=(kb == KB - 1))
                                # weave one RNN step every other kb batch
                                if kb % 2 == 0 and wi < len(weave):
                                    rnn_step(weave[wi])
                                    wi += 1
                            for dc in dcs:
                                st = stg.tile([mrows, NPC], F16, tag="st")
                                nc.vector.tensor_copy(st[:], psums[dc][:])
                                nc.sync.dma_start(
                                    a2a_in[r][dc * RJ + mbi * 128:
                                              dc * RJ + mbi * 128 + mrows, :],
                                    st[:])
                    assert wi == len(weave)
                    nc.gpsimd.collective_compute(
                        "AllToAll", mybir.AluOpType.bypass,
                        replica_groups=[list(range(NCORES))],
                        ins=[a2a_in[r].opt()], outs=[a2a_out[r].opt()])

            # ---- phase 3 tail: remaining RNN steps ----
            with nc.named_scope("rnn"):
                load_ubig(2, 0)
                load_ubig(2, 1)
                for b in range(64, 96):
                    rnn_step(b)
                load_ubig(3, 0)
                load_ubig(3, 1)
                for b in range(96, 128):
                    rnn_step(b)
                # final partial writeback (steps 126-127 not group-aligned)
                b0 = 126
                for h, (hts, oring) in enumerate(chains):
                    nc.sync.dma_start(
                        out[b0:b0 + 2, :, h * HALF:(h + 1) * HALF].rearrange(
                            "g j n -> j g n"),
                        oring[:, (b0 % OUT_RING) * HALF:
                              (b0 % OUT_RING + 2) * HALF].rearrange(
                            "j (g n) -> j g n", g=2))
            post3.__exit__(None, None, None)
            post2.__exit__(None, None, None)
            post.__exit__(None, None, None)
            post0b.__exit__(None, None, None)
            post0.__exit__(None, None, None)

    nc.compile()
    _PROGRAM_CACHE["nc"] = nc
    return nc


def _host_prep(x_in, edge_index, edge_weight, W, b, W_ih, W_hh, b_ih, b_hh, h0):
    """Build per-core input maps (all numpy, no device work)."""
    edge_index = np.asarray(edge_index).astype(np.int64)
    # exact reference remap: rank among unique ids (size=N, fill=2**30)
    uniq = np.unique(edge_index)
    if uniq.size < N:
        uniq = np.concatenate([uniq, np.full(N - uniq.size, 2 ** 30, np.int64)])
    else:
        uniq = uniq[:N]
    ei = np.searchsorted(uniq, edge_index)
    src, dst = ei[0], ei[1]

    ew = np.asarray(edge_weight, np.float64)
    deg = np.zeros(N, np.float64)
    np.add.at(deg, dst, ew)
    deg += 1.0  # self loops (weight 1)
    dinv = np.where(deg > 0, 1.0 / np.sqrt(deg), 0.0)

    AT = np.zeros((NP, NP), np.float32)
    np.add.at(AT, (src, dst), (dinv[src] * ew * dinv[dst]).astype(np.float32))
    idx = np.arange(N)
    AT[idx, idx] += (dinv * dinv).astype(np.float32)
    AT16 = AT.astype(np.float16)

    W = np.asarray(W, np.float32)
    W_ih = np.asarray(W_ih, np.float32)
    W2 = (W.astype(np.float64) @ W_ih.T.astype(np.float64)).astype(np.float16)
    c0 = (np.asarray(b, np.float32) @ W_ih.T + np.asarray(b_ih, np.float32)
          + np.asarray(b_hh, np.float32)).astype(np.float32).reshape(J, 1)
    wh = np.asarray(W_hh, np.float32).T.astype(np.float16)
    eye = np.eye(J, dtype=np.float16)

    x_in = np.asarray(x_in, np.float32)
    h0 = np.asarray(h0, np.float32)
    h0p = np.zeros((NP, J), np.float16)
    h0p[:N] = h0.astype(np.float16)

    in_maps = []
    for c in range(NCORES):
        samples = [32 * r + 4 * c + s4 for r in range(R) for s4 in range(SR)]
        xc = x_in[samples]                                # (S, N, F)
        xTc = np.ascontiguousarray(
            xc.transpose(0, 2, 1)).astype(np.float16)     # (S, F, N)
        h0Tc = np.ascontiguousarray(
            h0p[c * NPC:(c + 1) * NPC].T)                 # (J, NPC)
        in_maps.append({"xT": xTc, "at": AT16, "w2": W2, "wh": wh,
                        "eye": eye, "c0": c0, "h0T": h0Tc})
    return in_maps


def _assemble(results):
    parts = []
    for c in range(NCORES):
        o = results[c]["out"]                 # (B, J, NPC) fp16
        parts.append(np.ascontiguousarray(o.transpose(0, 2, 1)))  # (B, NPC, J)
    full = np.concatenate(parts, axis=1)      # (B, NP, J)
    return full[:, :N, :].astype(np.float32)


def run_internal(inputs, trace=False, trace_cores=None):
    nc = _build_program()
    in_maps = _host_prep(**inputs)
    res = run_bass_kernel_spmd(nc, in_maps, list(range(NCORES)), trace=trace,
                               trace_cores=trace_cores)
    return _assemble(res.results), res


def kernel(**inputs) -> np.ndarray:
    out, _ = run_internal(inputs, trace=False)
    return out


# revision 13
# speedup vs baseline: 1.7563x; 1.0381x over previous
"""Trainium2 Bass kernel for GCN+RNN (nn_GCNN_RNN_32461362823865).

Strategy:
  - Host: build dense normalized adjacency A^T (fp16, 3072-padded) from the
    edge list (exact reference remap semantics), fold W2 = W @ W_ih.T and
    c0 = b @ W_ih.T + b_ih + b_hh, pre-transpose/cast x_in per core.
  - Device phase 1 (batch-sharded, 16 samples/core in 4 rounds of 4):
      z = x @ W2                  (128->50, PE, fp16)
      U^T = (z^T stationary) x A^T streamed  -> (s*50+j, dst_node) f32 PSUM
      cast fp16 (DVE), write to per-round AllToAll input, dest-core-major.
  - 4 AllToAll rounds reshard U batch->node sharding (384 nodes/core);
    round r's collective overlaps round r+1's matmuls, and RNN steps for
    round r are woven into round r+2's matmul emission.
  - Phase 3 (node-sharded RNN): h^T (50, nodes) fp16; two node-chains
    (192 each) pipelined; per chain-step two matmuls (I x U accumulate
    W_hh^T x h) then ScalarE tanh with per-partition bias c0. Outputs
    staged in SBUF rings, written back 8 steps per DMA.
  - Host: concat per-core (B, 50, 384) outputs, transpose, crop, upcast.

  Sample->core map: core c, round r holds global samples 32*r + 4*c + [0..4).
"""
import numpy as np

import concourse.bacc as bacc
import concourse.mybir as mybir
from concourse import tile
from concourse.bass_utils import run_bass_kernel_spmd

# ---- problem constants (hardcoded per contract) ----
N = 3070          # nodes
NP = 3072         # padded nodes (24 * 128, 8 * 384)
F = 128           # input features
J = 50            # folded feature dim (= RNN hidden)
B = 128           # batch (RNN sequence length)
NCORES = 8
S = B // NCORES   # samples per core = 16
NPC = NP // NCORES  # nodes per core = 384
KB = NP // 128    # 24 contraction blocks
HALF = NPC // 2   # 192 nodes per RNN chain
R = 4             # a2a rounds
SR = S // R       # samples per round per core = 4
RJ = SR * J       # 200 U^T rows per round per core
MBS = [128, RJ - 128]   # M-block rows within a round
DCP = [list(range(6)), [6, 7]]  # dest-core passes (6+2 psum banks)

F16 = mybir.dt.float16
F32 = mybir.dt.float32
TANH = mybir.ActivationFunctionType.Tanh

_PROGRAM_CACHE = {}


def _build_program():
    if "nc" in _PROGRAM_CACHE:
        return _PROGRAM_CACHE["nc"]
    nc = bacc.Bacc("TRN2", target_bir_lowering=False, debug=False,
                   num_devices=NCORES)

    xT = nc.dram_tensor("xT", [S, F, N], F16, kind="ExternalInput")
    at = nc.dram_tensor("at", [NP, NP], F16, kind="ExternalInput")
    w2 = nc.dram_tensor("w2", [F, J], F16, kind="ExternalInput")
    wh = nc.dram_tensor("wh", [J, J], F16, kind="ExternalInput")
    eye = nc.dram_tensor("eye", [J, J], F16, kind="ExternalInput")
    c0 = nc.dram_tensor("c0", [J, 1], F32, kind="ExternalInput")
    h0T = nc.dram_tensor("h0T", [J, NPC], F16, kind="ExternalInput")
    out = nc.dram_tensor("out", [B, J, NPC], F16, kind="ExternalOutput")

    with tile.TileContext(nc) as tc:
        with (
            tc.tile_pool(name="consts", bufs=1) as consts,
            tc.tile_pool(name="persist", bufs=1) as persist,
            tc.tile_pool(name="dram", bufs=1, space="DRAM") as dram,
        ):
            w2_sb = consts.tile([F, J], F16, tag="w2_sb")
            wh_sb = consts.tile([J, J], F16, tag="wh_sb")
            eye_sb = consts.tile([J, J], F16, tag="eye_sb")
            c0_sb = consts.tile([J, 1], F32, tag="c0_sb")
            nc.sync.dma_start(w2_sb[:], w2[:])
            nc.sync.dma_start(wh_sb[:], wh[:])
            nc.sync.dma_start(eye_sb[:], eye[:])
            nc.sync.dma_start(c0_sb[:], c0[:])

            at_sb = persist.tile([128, KB * NP], F16, tag="at_sb")
            z_sb = persist.tile([128, KB * S * J], F16, tag="z_sb")
            SJ = S * J

            a2a_in = [dram.tile([NCORES * RJ, NPC], F16, name=f"a2ai_{r}")
                      for r in range(R)]
            a2a_out = [dram.tile([NCORES * RJ, NPC], F16, name=f"a2ao_{r}")
                       for r in range(R)]

            # ---- phase 1a: z[src,(s,j)] = x @ W2 (x loads first) ----
            with nc.named_scope("zphase"):
                with (
                    tc.tile_pool(name="xin", bufs=2) as xin,
                    tc.tile_pool(name="zpsum", bufs=6, space="PSUM") as zpsum,
                ):
                    for s in range(S):
                        xbig = xin.tile([F, NP], F16, tag="xbig")
                        nc.sync.dma_start(xbig[:, 0:N], xT[s])
                        nc.vector.memset(xbig[:, N:NP], 0.0)
                        for kb in range(KB):
                            zp = zpsum.tile([128, J], F32, tag="zp")
                            nc.tensor.matmul(
                                zp[:], xbig[:, kb * 128:(kb + 1) * 128],
                                w2_sb[:], start=True, stop=True)
                            nc.vector.tensor_copy(
                                z_sb[:, kb * SJ + s * J:
                                     kb * SJ + (s + 1) * J], zp[:])

            # A^T loads queue behind the x loads (needed from U-phase on)
            for kb in range(KB):
                nc.sync.dma_start(at_sb[:, kb * NP:(kb + 1) * NP],
                                  at[kb * 128:(kb + 1) * 128, :])

            # ---- RNN state (declared before weave) ----
            post0 = tc.tile_pool(name="upsum", bufs=6, space="PSUM")
            upsum = post0.__enter__()
            post0b = tc.tile_pool(name="p3psum", bufs=1, space="PSUM")
            p3psum = post0b.__enter__()
            post = tc.tile_pool(name="stg", bufs=3)
            stg = post.__enter__()
            post2 = tc.tile_pool(name="upool", bufs=2)
            upool = post2.__enter__()
            post3 = tc.tile_pool(name="p3", bufs=1)
            p3 = post3.__enter__()
            u_tiles = {}

            def load_ubig(r, c):
                u = upool.tile([J, SR * NPC], F16, tag="u",
                               name=f"ubig_{r}_{c}")
                u_tiles[(r, c)] = u
                nc.sync.dma_start(
                    u[:].rearrange("j (s n) -> j s n", s=SR),
                    a2a_out[r][c * RJ:(c + 1) * RJ, :].rearrange(
                        "(s j) n -> j s n", j=J))

            RING = 4
            OUT_RING = 12
            OGRP = 6
            chains = []
            for h in range(2):
                hts = [p3.tile([J, HALF], F16, name=f"ht_{h}_{r}")
                       for r in range(RING)]
                oring = p3.tile([J, OUT_RING * HALF], F16, name=f"oring_{h}")
                chains.append((hts, oring))
                nc.sync.dma_start(hts[0][:], h0T[:, h * HALF:(h + 1) * HALF])

            def rnn_step(b):
                r, c, s4 = b // 32, (b % 32) // 4, b % 4
                if s4 == 0 and c + 2 < NCORES:
                    load_ubig(r, c + 2)   # prefetch 2 source-cores ahead
                ring = b % RING
                slot = b % OUT_RING
                for h, (hts, oring) in enumerate(chains):
                    ut = u_tiles[(r, c)]
                    usrc = ut[:, s4 * NPC + h * HALF:
                              s4 * NPC + h * HALF + HALF]
                    pp = p3psum.tile([J, HALF], F32, tag=f"pp{h}",
                                     name=f"pp_{h}_{b}")
                    nc.tensor.matmul(pp[:], eye_sb[:], usrc,
                                     start=True, stop=False)
                    nc.tensor.matmul(pp[:], wh_sb[:], hts[ring][:],
                                     start=False, stop=True)
                    nxt = hts[(b + 1) % RING]
                    nc.scalar.activation(nxt[:], pp[:], TANH,
                                         bias=c0_sb[:, 0:1])
                    nc.vector.tensor_copy(
                        oring[:, slot * HALF:(slot + 1) * HALF], nxt[:])
                if slot % OGRP == OGRP - 1 and b >= OGRP - 1:
                    b0 = b - (OGRP - 1)
                    for h, (hts, oring) in enumerate(chains):
                        nc.sync.dma_start(
                            out[b0:b0 + OGRP, :,
                                h * HALF:(h + 1) * HALF].rearrange(
                                "g j n -> j g n"),
                            oring[:, (b0 % OUT_RING) * HALF:
                                  (b0 % OUT_RING + OGRP) * HALF].rearrange(
                                "j (g n) -> j g n", g=OGRP))

            # ---- phase 1b: U^T per round + collectives, weaving RNN ----
            with nc.named_scope("ummphase"):
                for r in range(R):
                    if r >= 2:  # u loads for round r-2 (a2a r-2 done by now)
                        load_ubig(r - 2, 0)
                        load_ubig(r - 2, 1)
                    weave = list(range(32 * (r - 2), 32 * (r - 2) + 32)) \
                        if r >= 2 else []
                    wi = 0
                    for mbi, mrows in enumerate(MBS):
                        row0 = r * RJ + mbi * 128  # start row in z cols
                        for pi, dcs in enumerate(DCP):
                            psums = {}
                            for dc in dcs:
                                psums[dc] = upsum.tile(
                                    [mrows, NPC], F32, tag="up",
                                    name=f"up_{r}_{mbi}_{dc}")
                            for kb in range(KB):
                                lhsT = z_sb[:, kb * SJ + row0:
                                            kb * SJ + row0 + mrows]
                                for dc in dcs:
                                    nc.tensor.matmul(
                                        psums[dc][:], lhsT,
                                        at_sb[:, kb * NP + dc * NPC:
                                              kb * NP + (dc + 1) * NPC],
                                        start=(kb == 0), stop=(kb == KB - 1))
                                # weave one RNN step every other kb batch
                                if kb % 2 == 0 and wi < len(weave):
                                    rnn_step(weave[wi])
                                    wi += 1
                            for dc in dcs:
                                st = stg.tile([mrows, NPC], F16, tag="st")
                                nc.vector.tensor_copy(st[:], psums[dc][:])
                                nc.sync.dma_start(
                                    a2a_in[r][dc * RJ + mbi * 128:
                                              dc * RJ + mbi * 128 + mrows, :],
                                    st[:])
                    assert wi == len(weave)
                    nc.gpsimd.collective_compute(
                        "AllToAll", mybir.AluOpType.bypass,
                        replica_groups=[list(range(NCORES))],
                        ins=[a2a_in[r].opt()], outs=[a2a_out[r].opt()])

            # ---- phase 3 tail: remaining RNN steps ----
            with nc.named_scope("rnn"):
                load_ubig(2, 0)
                load_ubig(2, 1)
                for b in range(64, 96):
                    rnn_step(b)
                load_ubig(3, 0)
                load_ubig(3, 1)
                for b in range(96, 128):
                    rnn_step(b)
                # final partial writeback (steps 126-127 not group-aligned)
                b0 = 126
                for h, (hts, oring) in enumerate(chains):
                    nc.sync.dma_start(
                        out[b0:b0 + 2, :, h * HALF:(h + 1) * HALF].rearrange(
                            "g j n -> j g n"),
                        oring[:, (b0 % OUT_RING) * HALF:
                              (b0 % OUT_RING + 2) * HALF].rearrange(
                            "j (g n) -> j g n", g=2))
            post3.__exit__(None, None, None)
            post2.__exit__(None, None, None)
            post.__exit__(None, None, None)
            post0b.__exit__(None, None, None)
            post0.__exit__(None, None, None)

    nc.compile()
    _PROGRAM_CACHE["nc"] = nc
    return nc


def _host_prep(x_in, edge_index, edge_weight, W, b, W_ih, W_hh, b_ih, b_hh, h0):
    """Build per-core input maps (all numpy, no device work)."""
    edge_index = np.asarray(edge_index).astype(np.int64)
    # exact reference remap: rank among unique ids (size=N, fill=2**30)
    uniq = np.unique(edge_index)
    if uniq.size < N:
        uniq = np.concatenate([uniq, np.full(N - uniq.size, 2 ** 30, np.int64)])
    else:
        uniq = uniq[:N]
    ei = np.searchsorted(uniq, edge_index)
    src, dst = ei[0], ei[1]

    ew = np.asarray(edge_weight, np.float64)
    deg = np.zeros(N, np.float64)
    np.add.at(deg, dst, ew)
    deg += 1.0  # self loops (weight 1)
    dinv = np.where(deg > 0, 1.0 / np.sqrt(deg), 0.0)

    AT = np.zeros((NP, NP), np.float32)
    np.add.at(AT, (src, dst), (dinv[src] * ew * dinv[dst]).astype(np.float32))
    idx = np.arange(N)
    AT[idx, idx] += (dinv * dinv).astype(np.float32)
    AT16 = AT.astype(np.float16)

    W = np.asarray(W, np.float32)
    W_ih = np.asarray(W_ih, np.float32)
    W2 = (W.astype(np.float64) @ W_ih.T.astype(np.float64)).astype(np.float16)
    c0 = (np.asarray(b, np.float32) @ W_ih.T + np.asarray(b_ih, np.float32)
          + np.asarray(b_hh, np.float32)).astype(np.float32).reshape(J, 1)
    wh = np.asarray(W_hh, np.float32).T.astype(np.float16)
    eye = np.eye(J, dtype=np.float16)

    x_in = np.asarray(x_in, np.float32)
    h0 = np.asarray(h0, np.float32)
    h0p = np.zeros((NP, J), np.float16)
    h0p[:N] = h0.astype(np.float16)

    in_maps = []
    for c in range(NCORES):
        samples = [32 * r + 4 * c + s4 for r in range(R) for s4 in range(SR)]
        xc = x_in[samples]                                # (S, N, F)
        xTc = np.ascontiguousarray(
            xc.transpose(0, 2, 1)).astype(np.float16)     # (S, F, N)
        h0Tc = np.ascontiguousarray(
            h0p[c * NPC:(c + 1) * NPC].T)                 # (J, NPC)
        in_maps.append({"xT": xTc, "at": AT16, "w2": W2, "wh": wh,
                        "eye": eye, "c0": c0, "h0T": h0Tc})
    return in_maps


def _assemble(results):
    parts = []
    for c in range(NCORES):
        o = results[c]["out"]                 # (B, J, NPC) fp16
        parts.append(np.ascontiguousarray(o.transpose(0, 2, 1)))  # (B, NPC, J)
    full = np.concatenate(parts, axis=1)      # (B, NP, J)
    return full[:, :N, :].astype(np.float32)


def run_internal(inputs, trace=False, trace_cores=None):
    nc = _build_program()
    in_maps = _host_prep(**inputs)
    res = run_bass_kernel_spmd(nc, in_maps, list(range(NCORES)), trace=trace,
                               trace_cores=trace_cores)
    return _assemble(res.results), res


def kernel(**inputs) -> np.ndarray:
    out, _ = run_internal(inputs, trace=False)
    return out


# revision 14
# speedup vs baseline: 1.8072x; 1.0290x over previous
"""Trainium2 Bass kernel for GCN+RNN (nn_GCNN_RNN_32461362823865).

Strategy:
  - Host: build dense normalized adjacency A^T (fp16, 3072-padded) from the
    edge list (exact reference remap semantics), fold W2 = W @ W_ih.T and
    c0 = b @ W_ih.T + b_ih + b_hh, pre-transpose/cast x_in per core.
  - Device phase 1 (batch-sharded, 16 samples/core in 4 rounds of 4):
      z = x @ W2                  (128->50, PE, fp16)
      U^T = (z^T stationary) x A^T streamed  -> (s*50+j, dst_node) f32 PSUM
      cast fp16 (DVE), write to per-round AllToAll input, dest-core-major.
  - 4 AllToAll rounds reshard U batch->node sharding (384 nodes/core);
    round r's collective overlaps round r+1's matmuls, and RNN steps for
    round r are woven into round r+2's matmul emission.
  - Phase 3 (node-sharded RNN): h^T (50, nodes) fp16; two node-chains
    (192 each) pipelined; per chain-step two matmuls (I x U accumulate
    W_hh^T x h) then ScalarE tanh with per-partition bias c0. Outputs
    staged in SBUF rings, written back 8 steps per DMA.
  - Host: concat per-core (B, 50, 384) outputs, transpose, crop, upcast.

  Sample->core map: core c, round r holds global samples 32*r + 4*c + [0..4).
"""
import numpy as np

import concourse.bacc as bacc
import concourse.mybir as mybir
from concourse import tile
from concourse.bass_utils import run_bass_kernel_spmd

# ---- problem constants (hardcoded per contract) ----
N = 3070          # nodes
NP = 3072         # padded nodes (24 * 128, 8 * 384)
F = 128           # input features
J = 50            # folded feature dim (= RNN hidden)
B = 128           # batch (RNN sequence length)
NCORES = 8
S = B // NCORES   # samples per core = 16
NPC = NP // NCORES  # nodes per core = 384
KB = NP // 128    # 24 contraction blocks
HALF = NPC // 2   # 192 nodes per RNN chain
R = 4             # a2a rounds
SR = S // R       # samples per round per core = 4
RJ = SR * J       # 200 U^T rows per round per core
MBS = [128, RJ - 128]   # M-block rows within a round
DCP = [list(range(6)), [6, 7]]  # dest-core passes (6+2 psum banks)

F16 = mybir.dt.float16
F32 = mybir.dt.float32
TANH = mybir.ActivationFunctionType.Tanh

_PROGRAM_CACHE = {}


def _build_program():
    if "nc" in _PROGRAM_CACHE:
        return _PROGRAM_CACHE["nc"]
    nc = bacc.Bacc("TRN2", target_bir_lowering=False, debug=False,
                   num_devices=NCORES)

    xT = nc.dram_tensor("xT", [S, F, N], F16, kind="ExternalInput")
    at = nc.dram_tensor("at", [NP, NP], F16, kind="ExternalInput")
    w2 = nc.dram_tensor("w2", [F, J], F16, kind="ExternalInput")
    wh = nc.dram_tensor("wh", [J, J], F16, kind="ExternalInput")
    eye = nc.dram_tensor("eye", [J, J], F16, kind="ExternalInput")
    c0 = nc.dram_tensor("c0", [J, 1], F32, kind="ExternalInput")
    h0T = nc.dram_tensor("h0T", [J, NPC], F16, kind="ExternalInput")
    out = nc.dram_tensor("out", [B, J, NPC], F16, kind="ExternalOutput")

    with tile.TileContext(nc) as tc:
        with (
            tc.tile_pool(name="consts", bufs=1) as consts,
            tc.tile_pool(name="persist", bufs=1) as persist,
            tc.tile_pool(name="dram", bufs=1, space="DRAM") as dram,
        ):
            w2_sb = consts.tile([F, J], F16, tag="w2_sb")
            wh_sb = consts.tile([J, J], F16, tag="wh_sb")
            eye_sb = consts.tile([J, J], F16, tag="eye_sb")
            c0_sb = consts.tile([J, 1], F32, tag="c0_sb")
            nc.sync.dma_start(w2_sb[:], w2[:])
            nc.sync.dma_start(wh_sb[:], wh[:])
            nc.sync.dma_start(eye_sb[:], eye[:])
            nc.sync.dma_start(c0_sb[:], c0[:])

            at_sb = persist.tile([128, KB * NP], F16, tag="at_sb")
            z_sb = persist.tile([128, KB * S * J], F16, tag="z_sb")
            SJ = S * J

            a2a_in = [dram.tile([NCORES * RJ, NPC], F16, name=f"a2ai_{r}")
                      for r in range(R)]
            a2a_out = [dram.tile([NCORES * RJ, NPC], F16, name=f"a2ao_{r}")
                       for r in range(R)]

            # ---- phase 1a: z[src,(s,j)] = x @ W2 (x loads first) ----
            with nc.named_scope("zphase"):
                with (
                    tc.tile_pool(name="xin", bufs=3) as xin,
                    tc.tile_pool(name="zpsum", bufs=6, space="PSUM") as zpsum,
                ):
                    for s in range(S):
                        xbig = xin.tile([F, NP], F16, tag="xbig")
                        nc.sync.dma_start(xbig[:, 0:N], xT[s])
                        nc.vector.memset(xbig[:, N:NP], 0.0)
                        for kb in range(KB):
                            zp = zpsum.tile([128, J], F32, tag="zp")
                            nc.tensor.matmul(
                                zp[:], xbig[:, kb * 128:(kb + 1) * 128],
                                w2_sb[:], start=True, stop=True)
                            nc.vector.tensor_copy(
                                z_sb[:, kb * SJ + s * J:
                                     kb * SJ + (s + 1) * J], zp[:])

            # A^T loads queue behind the x loads (needed from U-phase on)
            for kb in range(KB):
                nc.sync.dma_start(at_sb[:, kb * NP:(kb + 1) * NP],
                                  at[kb * 128:(kb + 1) * 128, :])

            # ---- RNN state (declared before weave) ----
            post0 = tc.tile_pool(name="upsum", bufs=6, space="PSUM")
            upsum = post0.__enter__()
            post0b = tc.tile_pool(name="p3psum", bufs=1, space="PSUM")
            p3psum = post0b.__enter__()
            post = tc.tile_pool(name="stg", bufs=3)
            stg = post.__enter__()
            post2 = tc.tile_pool(name="upool", bufs=2)
            upool = post2.__enter__()
            post3 = tc.tile_pool(name="p3", bufs=1)
            p3 = post3.__enter__()
            u_tiles = {}

            def load_ubig(r, c):
                u = upool.tile([J, SR * NPC], F16, tag="u",
                               name=f"ubig_{r}_{c}")
                u_tiles[(r, c)] = u
                nc.sync.dma_start(
                    u[:].rearrange("j (s n) -> j s n", s=SR),
                    a2a_out[r][c * RJ:(c + 1) * RJ, :].rearrange(
                        "(s j) n -> j s n", j=J))

            RING = 4
            OUT_RING = 12
            OGRP = 6
            chains = []
            for h in range(2):
                hts = [p3.tile([J, HALF], F16, name=f"ht_{h}_{r}")
                       for r in range(RING)]
                oring = p3.tile([J, OUT_RING * HALF], F16, name=f"oring_{h}")
                chains.append((hts, oring))
                nc.sync.dma_start(hts[0][:], h0T[:, h * HALF:(h + 1) * HALF])

            def rnn_step(b):
                r, c, s4 = b // 32, (b % 32) // 4, b % 4
                if s4 == 0 and c + 2 < NCORES:
                    load_ubig(r, c + 2)   # prefetch 2 source-cores ahead
                ring = b % RING
                slot = b % OUT_RING
                for h, (hts, oring) in enumerate(chains):
                    ut = u_tiles[(r, c)]
                    usrc = ut[:, s4 * NPC + h * HALF:
                              s4 * NPC + h * HALF + HALF]
                    pp = p3psum.tile([J, HALF], F32, tag=f"pp{h}",
                                     name=f"pp_{h}_{b}")
                    nc.tensor.matmul(pp[:], eye_sb[:], usrc,
                                     start=True, stop=False)
                    nc.tensor.matmul(pp[:], wh_sb[:], hts[ring][:],
                                     start=False, stop=True)
                    nxt = hts[(b + 1) % RING]
                    nc.scalar.activation(nxt[:], pp[:], TANH,
                                         bias=c0_sb[:, 0:1])
                    nc.vector.tensor_copy(
                        oring[:, slot * HALF:(slot + 1) * HALF], nxt[:])
                if slot % OGRP == OGRP - 1 and b >= OGRP - 1:
                    b0 = b - (OGRP - 1)
                    for h, (hts, oring) in enumerate(chains):
                        nc.sync.dma_start(
                            out[b0:b0 + OGRP, :,
                                h * HALF:(h + 1) * HALF].rearrange(
                                "g j n -> j g n"),
                            oring[:, (b0 % OUT_RING) * HALF:
                                  (b0 % OUT_RING + OGRP) * HALF].rearrange(
                                "j (g n) -> j g n", g=OGRP))

            # ---- phase 1b: U^T per round + collectives, weaving RNN ----
            with nc.named_scope("ummphase"):
                for r in range(R):
                    if r >= 2:  # u loads for round r-2 (a2a r-2 done by now)
                        load_ubig(r - 2, 0)
                        load_ubig(r - 2, 1)
                    weave = list(range(32 * (r - 2), 32 * (r - 2) + 32)) \
                        if r >= 2 else []
                    wi = 0
                    sidx = 0
                    for mbi, mrows in enumerate(MBS):
                        row0 = r * RJ + mbi * 128  # start row in z cols
                        for pi, dcs in enumerate(DCP):
                            psums = {}
                            for dc in dcs:
                                psums[dc] = upsum.tile(
                                    [mrows, NPC], F32, tag="up",
                                    name=f"up_{r}_{mbi}_{dc}")
                            for kb in range(KB):
                                lhsT = z_sb[:, kb * SJ + row0:
                                            kb * SJ + row0 + mrows]
                                for dc in dcs:
                                    nc.tensor.matmul(
                                        psums[dc][:], lhsT,
                                        at_sb[:, kb * NP + dc * NPC:
                                              kb * NP + (dc + 1) * NPC],
                                        start=(kb == 0), stop=(kb == KB - 1))
                                # weave RNN steps into the round's 2nd half
                                # (the a2a they consume completes ~a round
                                #  after its trigger)
                                sidx += 1
                                if sidx >= 32 and sidx % 2 == 0 \
                                        and wi < len(weave):
                                    rnn_step(weave[wi])
                                    wi += 1
                            for dc in dcs:
                                st = stg.tile([mrows, NPC], F16, tag="st")
                                nc.vector.tensor_copy(st[:], psums[dc][:])
                                nc.sync.dma_start(
                                    a2a_in[r][dc * RJ + mbi * 128:
                                              dc * RJ + mbi * 128 + mrows, :],
                                    st[:])
                    assert wi == len(weave)
                    nc.gpsimd.collective_compute(
                        "AllToAll", mybir.AluOpType.bypass,
                        replica_groups=[list(range(NCORES))],
                        ins=[a2a_in[r].opt()], outs=[a2a_out[r].opt()])

            # ---- phase 3 tail: remaining RNN steps ----
            with nc.named_scope("rnn"):
                load_ubig(2, 0)
                load_ubig(2, 1)
                for b in range(64, 96):
                    rnn_step(b)
                load_ubig(3, 0)
                load_ubig(3, 1)
                for b in range(96, 128):
                    rnn_step(b)
                # final partial writeback (steps 126-127 not group-aligned)
                b0 = 126
                for h, (hts, oring) in enumerate(chains):
                    nc.sync.dma_start(
                        out[b0:b0 + 2, :, h * HALF:(h + 1) * HALF].rearrange(
                            "g j n -> j g n"),
                        oring[:, (b0 % OUT_RING) * HALF:
                              (b0 % OUT_RING + 2) * HALF].rearrange(
                            "j (g n) -> j g n", g=2))
            post3.__exit__(None, None, None)
            post2.__exit__(None, None, None)
            post.__exit__(None, None, None)
            post0b.__exit__(None, None, None)
            post0.__exit__(None, None, None)

    nc.compile()
    _PROGRAM_CACHE["nc"] = nc
    return nc


def _host_prep(x_in, edge_index, edge_weight, W, b, W_ih, W_hh, b_ih, b_hh, h0):
    """Build per-core input maps (all numpy, no device work)."""
    edge_index = np.asarray(edge_index).astype(np.int64)
    # exact reference remap: rank among unique ids (size=N, fill=2**30)
    uniq = np.unique(edge_index)
    if uniq.size < N:
        uniq = np.concatenate([uniq, np.full(N - uniq.size, 2 ** 30, np.int64)])
    else:
        uniq = uniq[:N]
    ei = np.searchsorted(uniq, edge_index)
    src, dst = ei[0], ei[1]

    ew = np.asarray(edge_weight, np.float64)
    deg = np.zeros(N, np.float64)
    np.add.at(deg, dst, ew)
    deg += 1.0  # self loops (weight 1)
    dinv = np.where(deg > 0, 1.0 / np.sqrt(deg), 0.0)

    AT = np.zeros((NP, NP), np.float32)
    np.add.at(AT, (src, dst), (dinv[src] * ew * dinv[dst]).astype(np.float32))
    idx = np.arange(N)
    AT[idx, idx] += (dinv * dinv).astype(np.float32)
    AT16 = AT.astype(np.float16)

    W = np.asarray(W, np.float32)
    W_ih = np.asarray(W_ih, np.float32)
    W2 = (W.astype(np.float64) @ W_ih.T.astype(np.float64)).astype(np.float16)
    c0 = (np.asarray(b, np.float32) @ W_ih.T + np.asarray(b_ih, np.float32)
          + np.asarray(b_hh, np.float32)).astype(np.float32).reshape(J, 1)
    wh = np.asarray(W_hh, np.float32).T.astype(np.float16)
    eye = np.eye(J, dtype=np.float16)

    x_in = np.asarray(x_in, np.float32)
    h0 = np.asarray(h0, np.float32)
    h0p = np.zeros((NP, J), np.float16)
    h0p[:N] = h0.astype(np.float16)

    in_maps = []
    for c in range(NCORES):
        samples = [32 * r + 4 * c + s4 for r in range(R) for s4 in range(SR)]
        xc = x_in[samples]                                # (S, N, F)
        xTc = np.ascontiguousarray(
            xc.transpose(0, 2, 1)).astype(np.float16)     # (S, F, N)
        h0Tc = np.ascontiguousarray(
            h0p[c * NPC:(c + 1) * NPC].T)                 # (J, NPC)
        in_maps.append({"xT": xTc, "at": AT16, "w2": W2, "wh": wh,
                        "eye": eye, "c0": c0, "h0T": h0Tc})
    return in_maps


def _assemble(results):
    parts = []
    for c in range(NCORES):
        o = results[c]["out"]                 # (B, J, NPC) fp16
        parts.append(np.ascontiguousarray(o.transpose(0, 2, 1)))  # (B, NPC, J)
    full = np.concatenate(parts, axis=1)      # (B, NP, J)
    return full[:, :N, :].astype(np.float32)


def run_internal(inputs, trace=False, trace_cores=None):
    nc = _build_program()
    in_maps = _host_prep(**inputs)
    res = run_bass_kernel_spmd(nc, in_maps, list(range(NCORES)), trace=trace,
                               trace_cores=trace_cores)
    return _assemble(res.results), res


def kernel(**inputs) -> np.ndarray:
    out, _ = run_internal(inputs, trace=False)
    return out
